# revision 1
# baseline (speedup 1.0000x reference)
"""MaxSimilarity (cosine-sim row-max) Trainium2 kernel.

out[i] = max_j  (x1[i] . x2[j]) / max(||x1[i]|| * ||x2[j]||, 1e-8)
x1: [8192, 1024] f32, x2: [16384, 1024] f32, out: [8192] f32.

Strategy (8 NeuronCores):
- Shard x2 rows 8-way (2048 rows/core); replicate x1. Each core computes the
  row-max over its j-shard for all 8192 queries, scaled by 1/(n1*n2); host
  combines shards with elementwise max (max commutes with the positive
  per-row scale 1/n1).
- Matmul runs on the PE array in float32r (TF32: 8-bit exp / 11-bit mantissa)
  which streams at 1 cycle/row (4x faster than fp32). Full fp32 precision is
  recovered by splitting each operand into hi + lo TF32 parts on the host and
  accumulating hi1*hi2 + hi1*lo2 + lo1*hi2 into PSUM (lo1*lo2 ~ 2^-24,
  negligible). TERMS=1 selects plain TF32 (3x fewer matmuls, ~2e-5 absmax).
- Operands are pre-transposed/tiled on the host so every DMA is contiguous
  per partition; the contraction dim d lives on the partition axis.
- Row norms are computed on-device (ACT square+accumulate from the natural
  layout), refined to fp32 accuracy with two Babylonian iterations (ACT Sqrt
  alone has a loose ULP budget), inverted on DVE.
- PSUM tiles [128 q, 512 j] are drained on DVE: multiply by a partition-
  broadcast row of 1/n2, then reduce-max over j per j-block; the final
  per-query max is scaled by 1/n1 once at the end.
"""

import numpy as np

import concourse.bacc as bacc
import concourse.mybir as mybir
import concourse.tile as tile
from concourse.bass_utils import run_bass_kernel_spmd

N1, N2, D = 8192, 16384, 1024
P = 128
NCORES = 8
JS = N2 // NCORES          # 2048 j per core
JBLK = 512                 # psum moving free dim (one bank of fp32)
JB = JS // JBLK            # 4 psum blocks per core
M_TILES = N1 // P          # 64
K_TILES = D // P           # 8
J_TILES = JS // P          # 16
TERMS = 3                  # 3 = fp32-exact split, 1 = plain TF32

F32 = mybir.dt.float32
F32R = mybir.dt.float32r
AF = mybir.ActivationFunctionType
ALU = mybir.AluOpType
AX = mybir.AxisListType


def tf32_round(x):
    """Round fp32 to 11 explicit mantissa bits (RNE) = float32r-representable."""
    u = x.view(np.uint32).astype(np.uint64)
    keep = np.uint64(12)
    half = np.uint64(1 << 11)
    lsb = (u >> keep) & np.uint64(1)
    u2 = (u + half - np.uint64(1) + lsb) >> keep << keep
    return u2.astype(np.uint32).view(np.float32)


def _recip_norm(nc, pool, ss, w, tag):
    """recip = 1/sqrt(ss), fp32-accurate: ACT Sqrt seed + 2 Babylonian steps
    (all division via DVE iterative reciprocal)."""
    y = pool.tile([P, w], F32, tag=tag + "y")
    nc.scalar.activation(y[:], ss[:], AF.Sqrt)
    for it in range(2):
        r = pool.tile([P, w], F32, tag=tag + "r")
        nc.vector.reciprocal(r[:], y[:])
        t = pool.tile([P, w], F32, tag=tag + "t")
        nc.vector.tensor_tensor(t[:], ss[:], r[:], ALU.mult)       # ss / y
        y2 = pool.tile([P, w], F32, tag=tag + "y2")
        nc.vector.tensor_tensor(y2[:], y[:], t[:], ALU.add)        # y + ss/y
        nc.vector.tensor_scalar_mul(y2[:], y2[:], 0.5)
        y = y2
    out = pool.tile([P, w], F32, tag=tag + "o")
    nc.vector.reciprocal(out[:], y[:])
    return out


def build_nc(terms=TERMS):
    nc = bacc.Bacc(trn_type="TRN2")
    split = terms >= 2

    x1t_hi = nc.dram_tensor("x1t_hi", [M_TILES, P, K_TILES, P], F32R, kind="ExternalInput")
    x2t_hi = nc.dram_tensor("x2t_hi", [P, K_TILES, JS], F32R, kind="ExternalInput")
    if split:
        x1t_lo = nc.dram_tensor("x1t_lo", [M_TILES, P, K_TILES, P], F32R, kind="ExternalInput")
        x2t_lo = nc.dram_tensor("x2t_lo", [P, K_TILES, JS], F32R, kind="ExternalInput")
    x1n = nc.dram_tensor("x1n", [M_TILES, P, D], F32, kind="ExternalInput")
    x2n = nc.dram_tensor("x2n", [J_TILES, P, D], F32, kind="ExternalInput")
    out = nc.dram_tensor("out", [N1], F32, kind="ExternalOutput")

    with tile.TileContext(nc) as tc:
        with (
            tc.tile_pool(name="resident", bufs=1) as res,
            tc.tile_pool(name="stream", bufs=2) as stream,
            tc.tile_pool(name="scratch", bufs=2) as scr,
            tc.tile_pool(name="psum", bufs=8, space="PSUM") as psum,
        ):
            # ---- resident transposed x2 shard (hi now, lo after the n2
            # chain so the drain's bcast dependency is ready early) ----
            x2th_t = res.tile([P, K_TILES, JS], F32R, tag="x2th")
            nc.sync.dma_start(out=x2th_t[:], in_=x2t_hi[:])

            # ---- n2: sum of squares per x2 row, then 1/sqrt ----
            ss2 = res.tile([P, J_TILES], F32, tag="ss2")
            for t in range(J_TILES):
                xt = stream.tile([P, D], F32, tag="xnat")
                nc.sync.dma_start(out=xt[:], in_=x2n[t])
                sq = scr.tile([P, D], F32, tag="sqscr")
                nc.scalar.activation(sq[:], xt[:], AF.Square, accum_out=ss2[:, t : t + 1])
            recip_n2 = _recip_norm(nc, scr, ss2, J_TILES, "n2")

            # recip_n2 [P, J_TILES] (j = t*128+p) -> row [1, JS] -> bcast [P, JS]
            row = res.tile([1, JS], F32, tag="row")
            for t in range(J_TILES):
                nc.sync.dma_start(
                    out=row[:, t * P : (t + 1) * P], in_=recip_n2[:, t : t + 1]
                )
            bcast = res.tile([P, JS], F32, tag="bcast")
            nc.gpsimd.partition_broadcast(bcast[:], row[:])

            if split:
                x2tl_t = res.tile([P, K_TILES, JS], F32R, tag="x2tl")
                nc.sync.dma_start(out=x2tl_t[:], in_=x2t_lo[:])

            # ---- n1: sums of squares (before the loop; concurrent norm
            # traffic measurably slows the matmul stream if interleaved) ----
            ss1 = res.tile([P, M_TILES], F32, tag="ss1")
            for m in range(M_TILES):
                xt = stream.tile([P, D], F32, tag="xnat")
                nc.sync.dma_start(out=xt[:], in_=x1n[m])
                sq = scr.tile([P, D], F32, tag="sqscr")
                nc.scalar.activation(sq[:], xt[:], AF.Square, accum_out=ss1[:, m : m + 1])

            # ---- main loop: per 128-query block ----
            rmax_all = res.tile([P, M_TILES], F32, tag="rmaxall")
            for m in range(M_TILES):
                a_hi = stream.tile([P, K_TILES, P], F32R, tag="ahi")
                nc.sync.dma_start(out=a_hi[:], in_=x1t_hi[m])
                if split:
                    a_lo = stream.tile([P, K_TILES, P], F32R, tag="alo")
                    nc.sync.dma_start(out=a_lo[:], in_=x1t_lo[m])
                cmax = scr.tile([P, JB], F32, tag="cmax")
                for jb in range(JB):
                    js = slice(jb * JBLK, (jb + 1) * JBLK)
                    ps = psum.tile([P, JBLK], F32, tag="ps")
                    # hi*hi first so the first psum group doesn't wait on the
                    # x2t_lo resident DMA right behind x2t_hi in the queue
                    n_mm = K_TILES * terms
                    i_mm = 0
                    for k in range(K_TILES):
                        nc.tensor.matmul(
                            ps[:], a_hi[:, k, :], x2th_t[:, k, js],
                            start=(i_mm == 0), stop=(i_mm == n_mm - 1),
                        )
                        i_mm += 1
                    if terms >= 2:
                        for k in range(K_TILES):
                            nc.tensor.matmul(
                                ps[:], a_hi[:, k, :], x2tl_t[:, k, js],
                                start=False, stop=(i_mm == n_mm - 1),
                            )
                            i_mm += 1
                    if terms >= 3:
                        for k in range(K_TILES):
                            nc.tensor.matmul(
                                ps[:], a_lo[:, k, :], x2th_t[:, k, js],
                                start=False, stop=(i_mm == n_mm - 1),
                            )
                            i_mm += 1
                    ttr = scr.tile([P, JBLK], F32, tag="ttr")
                    nc.vector.tensor_tensor(ttr[:], ps[:], bcast[:, js], ALU.mult)
                    nc.vector.tensor_reduce(
                        cmax[:, jb : jb + 1], ttr[:], axis=AX.X, op=ALU.max
                    )
                nc.vector.tensor_reduce(
                    rmax_all[:, m : m + 1], cmax[:], axis=AX.X, op=ALU.max
                )

            # ---- n1 finish + final scale ----
            recip_n1 = _recip_norm(nc, scr, ss1, M_TILES, "n1")
            outsb = res.tile([P, M_TILES], F32, tag="outsb")
            nc.vector.tensor_tensor(outsb[:], rmax_all[:], recip_n1[:], ALU.mult)
            nc.sync.dma_start(out=out[:].rearrange("(m p) -> p m", p=P), in_=outsb[:])

    nc.finalize()
    return nc


_cache = {}


def _get_nc(terms):
    if terms not in _cache:
        _cache[terms] = build_nc(terms)
    return _cache[terms]


def _prep_inputs(x1, x2, terms):
    """Host-side layout prep: transpose + tile + TF32 hi/lo split + shard."""
    x1 = np.ascontiguousarray(x1, dtype=np.float32)
    x2 = np.ascontiguousarray(x2, dtype=np.float32)
    split = terms >= 2

    def tile_t(a, m_tiles):  # [R, D] -> [m, dp, k, q] with a[m*128+q, k*128+dp]
        r = a.shape[0]
        return np.ascontiguousarray(
            a.reshape(m_tiles, P, K_TILES, P).transpose(0, 3, 2, 1)
        )

    x1_hi = tf32_round(x1)
    x1t_hi = tile_t(x1_hi, M_TILES)
    if split:
        x1_lo = tf32_round(x1 - x1_hi)
        x1t_lo = tile_t(x1_lo, M_TILES)
    x1n = np.ascontiguousarray(x1.reshape(M_TILES, P, D))

    x2_hi = tf32_round(x2)
    if split:
        x2_lo = tf32_round(x2 - x2_hi)

    in_maps = []
    for c in range(NCORES):
        sl = slice(c * JS, (c + 1) * JS)

        def shard_t(a):  # [JS, D] -> [dp, k, j] with a[j, k*128+dp]
            return np.ascontiguousarray(
                a[sl].T.reshape(K_TILES, P, JS).transpose(1, 0, 2)
            )

        m = {
            "x1t_hi": x1t_hi,
            "x1n": x1n,
            "x2t_hi": shard_t(x2_hi),
            "x2n": np.ascontiguousarray(x2[sl].reshape(J_TILES, P, D)),
        }
        if split:
            m["x1t_lo"] = x1t_lo
            m["x2t_lo"] = shard_t(x2_lo)
        in_maps.append(m)
    return in_maps


def run(x1, x2, terms=TERMS, trace=False):
    nc = _get_nc(terms)
    in_maps = _prep_inputs(x1, x2, terms)
    res = run_bass_kernel_spmd(nc, in_maps, core_ids=list(range(NCORES)), trace=trace)
    parts = [res.results[c]["out"] for c in range(NCORES)]
    out = np.maximum.reduce(parts).astype(np.float32)
    return out, res


def kernel(x1, x2):
    out, _ = run(np.asarray(x1), np.asarray(x2), terms=TERMS, trace=False)
    return out



# revision 2
# speedup vs baseline: 2.8731x; 2.8731x over previous
"""MaxSimilarity (cosine-sim row-max) Trainium2 kernel.

out[i] = max_j  (x1[i] . x2[j]) / max(||x1[i]|| * ||x2[j]||, 1e-8)
x1: [8192, 1024] f32, x2: [16384, 1024] f32, out: [8192] f32.

Strategy (8 NeuronCores):
- Host pre-normalizes both matrices row-wise (norms are ~32 for randn rows,
  so the eps guard is never active) and rounds to TF32. The device kernel is
  then a pure GEMM + row-max: sim == x1n @ x2n.T, out = max over j.
- Shard x2 rows 8-way (2048 rows/core); replicate x1. Each core computes the
  row-max over its j-shard for all 8192 queries; host combines shards with
  elementwise max.
- Matmul runs in float32r (TF32: 11-bit mantissa, 1 cycle/row on the PE).
  Single-term TF32 on unit-norm rows gives ~1e-4 relative error on the
  row-max, far inside the tolerance; no hi/lo split needed.
- Operands are pre-transposed/tiled on the host so the contraction dim d is
  on the partition axis and every DMA line is contiguous.
- Per 128-query block: 4 psum banks accumulate 4 j-blocks of 512 over the
  8 k-tiles; one DVE reduce-max drains all 2048 j in a single instruction.
"""

import numpy as np

import concourse.bacc as bacc
import concourse.mybir as mybir
import concourse.tile as tile
from concourse.bass_utils import run_bass_kernel_spmd

N1, N2, D = 8192, 16384, 1024
P = 128
NCORES = 8
JS = N2 // NCORES          # 2048 j per core
JBLK = 512                 # one psum bank of fp32
JB = JS // JBLK            # 4 psum banks per m-tile
M_TILES = N1 // P          # 64
K_TILES = D // P           # 8

F32 = mybir.dt.float32
F32R = mybir.dt.float32r
ALU = mybir.AluOpType
AX = mybir.AxisListType


def tf32_round(x):
    """Round fp32 to 11 explicit mantissa bits (RNE) = float32r-representable."""
    u = x.view(np.uint32).astype(np.uint64)
    keep = np.uint64(12)
    half = np.uint64(1 << 11)
    lsb = (u >> keep) & np.uint64(1)
    u2 = (u + half - np.uint64(1) + lsb) >> keep << keep
    return u2.astype(np.uint32).view(np.float32)


def build_nc(k_outer=True):
    nc = bacc.Bacc(trn_type="TRN2")

    x1t = nc.dram_tensor("x1t", [M_TILES, P, K_TILES, P], F32R, kind="ExternalInput")
    x2t = nc.dram_tensor("x2t", [P, K_TILES, JS], F32R, kind="ExternalInput")
    out = nc.dram_tensor("out", [N1], F32, kind="ExternalOutput")

    with tile.TileContext(nc) as tc:
        with (
            tc.tile_pool(name="resident", bufs=1) as res,
            tc.tile_pool(name="stream", bufs=4) as stream,
            tc.tile_pool(name="psum", bufs=2, space="PSUM") as psum,
        ):
            # resident x2 shard, DMA'd in 4 j-chunks so the first m-tile's
            # matmuls can start before the whole 8MB lands
            x2s = res.tile([P, K_TILES, JS], F32R, tag="x2s")
            for jb in range(JB):
                js = slice(jb * JBLK, (jb + 1) * JBLK)
                nc.sync.dma_start(out=x2s[:, :, js], in_=x2t[:, :, js])

            rmax = res.tile([P, M_TILES], F32, tag="rmax")
            for m in range(M_TILES):
                a = stream.tile([P, K_TILES, P], F32R, tag="a")
                nc.sync.dma_start(out=a[:], in_=x1t[m])
                ps = psum.tile([P, JS], F32, tag="ps")  # 4 banks
                if k_outer:
                    # same stationary operand for 4 consecutive matmuls
                    for k in range(K_TILES):
                        for jb in range(JB):
                            js = slice(jb * JBLK, (jb + 1) * JBLK)
                            nc.tensor.matmul(
                                ps[:, js], a[:, k, :], x2s[:, k, js],
                                start=(k == 0), stop=(k == K_TILES - 1),
                            )
                else:
                    for jb in range(JB):
                        js = slice(jb * JBLK, (jb + 1) * JBLK)
                        for k in range(K_TILES):
                            nc.tensor.matmul(
                                ps[:, js], a[:, k, :], x2s[:, k, js],
                                start=(k == 0), stop=(k == K_TILES - 1),
                            )
                nc.vector.tensor_reduce(
                    rmax[:, m : m + 1], ps[:], axis=AX.X, op=ALU.max
                )

            nc.sync.dma_start(out=out[:].rearrange("(m p) -> p m", p=P), in_=rmax[:])

    nc.finalize()
    return nc


_cache = {}


def _get_nc(k_outer=True):
    key = ("v2", k_outer)
    if key not in _cache:
        _cache[key] = build_nc(k_outer)
    return _cache[key]


def _prep_inputs(x1, x2):
    """Host-side prep: row-normalize, TF32-round, transpose + tile, shard."""
    x1 = np.ascontiguousarray(x1, dtype=np.float32)
    x2 = np.ascontiguousarray(x2, dtype=np.float32)

    n1 = np.sqrt(np.einsum("ij,ij->i", x1, x1, dtype=np.float64))
    n2 = np.sqrt(np.einsum("ij,ij->i", x2, x2, dtype=np.float64))
    x1n = tf32_round((x1 / np.maximum(n1, 1e-8)[:, None]).astype(np.float32))
    x2n = tf32_round((x2 / np.maximum(n2, 1e-8)[:, None]).astype(np.float32))

    # x1t[m, dp, k, q] = x1n[m*128+q, k*128+dp]
    x1t = np.ascontiguousarray(
        x1n.reshape(M_TILES, P, K_TILES, P).transpose(0, 3, 2, 1)
    )

    in_maps = []
    for c in range(NCORES):
        sl = slice(c * JS, (c + 1) * JS)
        # x2t[dp, k, j] = x2n[sl][j, k*128+dp]
        x2tc = np.ascontiguousarray(
            x2n[sl].T.reshape(K_TILES, P, JS).transpose(1, 0, 2)
        )
        in_maps.append({"x1t": x1t, "x2t": x2tc})
    return in_maps


def run(x1, x2, k_outer=True, trace=False):
    nc = _get_nc(k_outer)
    in_maps = _prep_inputs(x1, x2)
    res = run_bass_kernel_spmd(nc, in_maps, core_ids=list(range(NCORES)), trace=trace)
    parts = [res.results[c]["out"] for c in range(NCORES)]
    out = np.maximum.reduce(parts).astype(np.float32)
    return out, res


def kernel(x1, x2):
    out, _ = run(np.asarray(x1), np.asarray(x2), trace=False)
    return out


# revision 3
# speedup vs baseline: 3.2244x; 1.1223x over previous
"""MaxSimilarity (cosine-sim row-max) Trainium2 kernel.

out[i] = max_j  (x1[i] . x2[j]) / max(||x1[i]|| * ||x2[j]||, 1e-8)
x1: [8192, 1024] f32, x2: [16384, 1024] f32, out: [8192] f32.

Strategy (8 NeuronCores):
- Host pre-normalizes both matrices row-wise (norms are ~32 for randn rows,
  so the eps guard is never active). The device kernel is then a pure
  GEMM + row-max: sim == x1n @ x2n.T, out = max over j.
- Shard x2 rows 8-way (2048 rows/core); replicate x1. Each core computes the
  row-max over its j-shard for all 8192 queries; host combines shards with
  elementwise max.
- Matmul operands are fp16 (1 cycle/row on the PE, like TF32, but half the
  HBM traffic and fast-weight-load). Unit-norm rows have elements ~N(0,
  1/1024) — wholly inside fp16 range; measured row-max error is ~2e-4
  relative, far inside tolerance. DTYPE="f32r" switches to TF32 operands
  (~9e-5 relative) at double the DMA bytes.
- Operands are pre-transposed/tiled on the host so the contraction dim d is
  on the partition axis and every DMA line is contiguous.
- Per 128-query block: 4 psum banks accumulate 4 j-blocks of 512 over the
  8 k-tiles (k-outer order, so the resident x2 shard can be DMA'd in
  k-chunks and matmuls start before the full shard lands); one DVE
  reduce-max drains all 2048 j in a single instruction.
- Output stays in the natural [partition, m-tile] layout (contiguous DMA);
  the host undoes the tiling. A transposed on-device DMA would scatter 8192
  4-byte words into HBM (~24us of descriptor drain).
"""

import numpy as np

import concourse.bacc as bacc
import concourse.mybir as mybir
import concourse.tile as tile
from concourse.bass_utils import run_bass_kernel_spmd

N1, N2, D = 8192, 16384, 1024
P = 128
NCORES = 8
JS = N2 // NCORES          # 2048 j per core
JBLK = 512                 # one psum bank of fp32
JB = JS // JBLK            # 4 psum banks per m-tile
M_TILES = N1 // P          # 64
K_TILES = D // P           # 8
DTYPE = "fp16"             # "fp16" | "f32r"

F32 = mybir.dt.float32
ALU = mybir.AluOpType
AX = mybir.AxisListType

_MM_DT = {"fp16": mybir.dt.float16, "f32r": mybir.dt.float32r}
_NP_DT = {"fp16": np.float16, "f32r": np.float32}


def tf32_round(x):
    """Round fp32 to 11 explicit mantissa bits (RNE) = float32r-representable."""
    u = x.view(np.uint32).astype(np.uint64)
    keep = np.uint64(12)
    half = np.uint64(1 << 11)
    lsb = (u >> keep) & np.uint64(1)
    u2 = (u + half - np.uint64(1) + lsb) >> keep << keep
    return u2.astype(np.uint32).view(np.float32)


def build_nc(dtype=DTYPE):
    nc = bacc.Bacc(trn_type="TRN2")
    mdt = _MM_DT[dtype]

    x1t = nc.dram_tensor("x1t", [M_TILES, P, K_TILES, P], mdt, kind="ExternalInput")
    x2t = nc.dram_tensor("x2t", [P, K_TILES, JS], mdt, kind="ExternalInput")
    out = nc.dram_tensor("out", [P, M_TILES], F32, kind="ExternalOutput")

    with tile.TileContext(nc) as tc:
        with (
            tc.tile_pool(name="resident", bufs=1) as res,
            tc.tile_pool(name="stream", bufs=4) as stream,
            tc.tile_pool(name="psum", bufs=2, space="PSUM") as psum,
        ):
            # resident x2 shard, DMA'd in k-chunks: the m-loop consumes k in
            # order, so the first matmuls only need chunk 0
            x2s = res.tile([P, K_TILES, JS], mdt, tag="x2s")
            for k in range(K_TILES):
                nc.sync.dma_start(out=x2s[:, k, :], in_=x2t[:, k, :])

            rmax = res.tile([P, M_TILES], F32, tag="rmax")
            for m in range(M_TILES):
                a = stream.tile([P, K_TILES, P], mdt, tag="a")
                nc.sync.dma_start(out=a[:], in_=x1t[m])
                ps = psum.tile([P, JS], F32, tag="ps")  # 4 banks
                for k in range(K_TILES):
                    for jb in range(JB):
                        js = slice(jb * JBLK, (jb + 1) * JBLK)
                        nc.tensor.matmul(
                            ps[:, js], a[:, k, :], x2s[:, k, js],
                            start=(k == 0), stop=(k == K_TILES - 1),
                        )
                nc.vector.tensor_reduce(
                    rmax[:, m : m + 1], ps[:], axis=AX.X, op=ALU.max
                )

            nc.sync.dma_start(out=out[:], in_=rmax[:])

    nc.finalize()
    return nc


_cache = {}


def _get_nc(dtype=DTYPE):
    key = ("v3", dtype)
    if key not in _cache:
        _cache[key] = build_nc(dtype)
    return _cache[key]


def _prep_inputs(x1, x2, dtype):
    """Host-side prep: row-normalize, round, transpose + tile, shard."""
    x1 = np.ascontiguousarray(x1, dtype=np.float32)
    x2 = np.ascontiguousarray(x2, dtype=np.float32)

    n1 = np.sqrt(np.einsum("ij,ij->i", x1, x1, dtype=np.float64))
    n2 = np.sqrt(np.einsum("ij,ij->i", x2, x2, dtype=np.float64))
    x1n = (x1 / np.maximum(n1, 1e-8)[:, None]).astype(np.float32)
    x2n = (x2 / np.maximum(n2, 1e-8)[:, None]).astype(np.float32)
    if dtype == "f32r":
        x1n, x2n = tf32_round(x1n), tf32_round(x2n)
    else:
        x1n, x2n = x1n.astype(np.float16), x2n.astype(np.float16)

    # x1t[m, dp, k, q] = x1n[m*128+q, k*128+dp]
    x1t = np.ascontiguousarray(
        x1n.reshape(M_TILES, P, K_TILES, P).transpose(0, 3, 2, 1)
    )

    in_maps = []
    for c in range(NCORES):
        sl = slice(c * JS, (c + 1) * JS)
        # x2t[dp, k, j] = x2n[sl][j, k*128+dp]
        x2tc = np.ascontiguousarray(
            x2n[sl].T.reshape(K_TILES, P, JS).transpose(1, 0, 2)
        )
        in_maps.append({"x1t": x1t, "x2t": x2tc})
    return in_maps


def run(x1, x2, dtype=DTYPE, trace=False):
    nc = _get_nc(dtype)
    in_maps = _prep_inputs(x1, x2, dtype)
    res = run_bass_kernel_spmd(nc, in_maps, core_ids=list(range(NCORES)), trace=trace)
    # out[p, m] holds the row-max of query m*128+p over this core's j-shard
    parts = [res.results[c]["out"].T.reshape(-1) for c in range(NCORES)]
    out = np.maximum.reduce(parts).astype(np.float32)
    return out, res


def kernel(x1, x2):
    out, _ = run(np.asarray(x1), np.asarray(x2), trace=False)
    return out
